# revision 1
# baseline (speedup 1.0000x reference)
"""Causal single-head attention on 8 TRN2 NeuronCores.

Problem: x [4, 2048, 768] f32; Wq/Wk/Wv [768, 768] f32 (torch Linear layout).
  q/k/v = x @ W.T ; scores = q k^T causal-masked; attn = softmax(scores/sqrt(768));
  out = attn @ v.

Sharding: core c -> batch b = c//2, half h = c%2. The two cores of a batch
split the 16 query tiles (128 rows each) INTERLEAVED: core h owns global
q-tiles {2*lt + h : lt in 0..7}. Causal attention for global q-tile g only
needs keys 0 .. 128*(g+1), i.e. ceil((g+1)/4) 512-wide key chunks; with the
even/odd interleave both cores see the identical chunk-count sequence
[1,1,2,2,3,3,4,4], so the SPMD program is uniform across cores while doing
EXACT causal work (no fully-masked chunks are ever computed). Only the
diagonal chunk of each q-tile needs masking; its 4 possible within-chunk
patterns are passed as a small per-core strip input ([128, 1024], window
picked by lt%2).

The host passes x^T (global key order, shared by the pair), xq^T (the core's
own interleaved query rows), and W^T - host transposes are pure layout prep.
Matmuls run in float32r (TensorE fast-fp32, 4x the fp32 rate at free dim
>= 256, ~2e-4 input rounding); raw fp32 bits feed float32r-typed DRAM inputs
directly - the PE converts on load, so the device does no transposes and no
rounding work at all.

Device pipeline per core:
  1. Q^T [768,1024] projected first (resident in SBUF), then stream x^T in
     512-col chunks -> K^T [768,2048] and V [2048,768] resident in SBUF;
     attention tiles can begin as soon as the first K/V chunks land.
  2. Per local q-tile lt (Nc = [1,1,2,2,3,3,4,4][lt] key chunks): scores via
     f32r matmuls; non-diagonal chunks exp directly from PSUM on ScalarE;
     the diagonal chunk gets a VectorE strip-add (fused evacuation) then exp;
     every exp emits its row-sum via accum_out (no max-subtraction: scaled
     scores are O(+-5), safely inside fp32 exp range). attn tiles transposed
     on TensorE 4-per-PSUM-bank; context accumulates over 4*Nc key tiles;
     softmax 1/rowsum is fused into the context PSUM evacuation.
"""

import os
import sys
from contextlib import ExitStack

import numpy as np

for _p in ("/opt/trn_rl_repo", "/root/.axon_site/_ro/trn_rl_repo"):
    if os.path.isdir(_p) and _p not in sys.path:
        sys.path.append(_p)

import concourse.mybir as mybir  # noqa: E402
import concourse.tile as tile  # noqa: E402
from concourse import bacc  # noqa: E402
from concourse.bass_utils import run_bass_kernel_spmd  # noqa: E402
from concourse.masks import make_identity  # noqa: E402

F32 = mybir.dt.float32
F32R = mybir.dt.float32r

BATCH = 4
SEQ = 2048
D = 768
DK = D // 128  # contraction chunks (6)
NQ = 1024  # query rows per core
LT = NQ // 128  # local q-tiles per core (8)
XC = 512  # streaming chunk width
NCS = [1, 1, 2, 2, 3, 3, 4, 4]  # key chunks per local q-tile (both cores!)
SCALE = 1.0 / float(np.sqrt(np.float32(D)))
NEG = -1e30

_CACHE = {}


def _build(repeat=1):
    nc = bacc.Bacc("TRN2", target_bir_lowering=False, debug=False, num_devices=8)
    xt_d = nc.declare_dram_parameter("xt", [D, SEQ], F32R, isOutput=False)
    xqt_d = nc.declare_dram_parameter("xqt", [D, NQ], F32R, isOutput=False)
    wqt_d = nc.declare_dram_parameter("wqt", [D, D], F32R, isOutput=False)
    wkt_d = nc.declare_dram_parameter("wkt", [D, D], F32R, isOutput=False)
    wvt_d = nc.declare_dram_parameter("wvt", [D, D], F32R, isOutput=False)
    strip_d = nc.declare_dram_parameter("strip", [128, 1024], F32, isOutput=False)
    out_d = nc.declare_dram_parameter("out", [NQ, D], F32, isOutput=True)

    # Rotate input DMAs across engines' DGE queues - a single queue serializes
    # the ~16MB of input transfers and stalls the PE at kernel start.
    _dma_i = [0]

    def dma_in(dst, src):
        eng = (nc.sync, nc.scalar)[_dma_i[0] % 2]
        eng.dma_start(dst, src)
        _dma_i[0] += 1

    # Round-robin PSUM evacuation between VectorE and ScalarE.
    _evac_i = [0]

    def evac(dst, src):
        if _evac_i[0] % 2 == 0:
            nc.vector.tensor_copy(dst, src)
        else:
            nc.scalar.copy(dst, src)
        _evac_i[0] += 1

    with tile.TileContext(nc) as tc, ExitStack() as ctx:
        persist = ctx.enter_context(tc.tile_pool(name="persist", bufs=1))

        ident = persist.tile([128, 128], F32)
        make_identity(nc, ident[:])

        strip = persist.tile([128, 1024], F32)
        nc.gpsimd.dma_start(strip[:], strip_d[:])

        kt = persist.tile([128, DK, SEQ], F32R)  # K^T
        vt = persist.tile([128, SEQ // 128, D], F32R)  # V (natural layout)
        qt_sb = persist.tile([128, DK, NQ], F32R)  # Q^T (resident)

        for _rep in range(repeat):
          # wk prefetches in a pool coexisting with the whole Q phase, so its
          # DMA is not blocked on the Q-phase SBUF region being released.
          with ExitStack() as p1:
            wkpool = p1.enter_context(tc.tile_pool(name="wkpool", bufs=1))
            wtk = wkpool.tile([128, DK, D], F32R, name="wtk")

            # ---------------- Phase 1b: Q^T projection (resident) ----------------
            with ExitStack() as p2b:
                wqpool = p2b.enter_context(tc.tile_pool(name="wqpool", bufs=1))
                xqc_p = p2b.enter_context(tc.tile_pool(name="xqc", bufs=2))
                ps_q = p2b.enter_context(
                    tc.tile_pool(name="ps_q", bufs=3, space="PSUM")
                )
                # wtq/xqc split into ko-halves as SEPARATE tiles: dependency
                # tracking is per-tile, so ko 0..2 matmuls start after half the
                # input bytes instead of waiting for the full load.
                wtqh = []
                for half in range(2):
                    wq_h = wqpool.tile([128, 3, D], F32R, name=f"wtq{half}")
                    dma_in(
                        wq_h[:],
                        wqt_d[half * 384 : (half + 1) * 384, :].rearrange(
                            "(ko p) o -> p ko o", p=128
                        ),
                    )
                    wtqh.append(wq_h)
                for sc in range(NQ // XC):
                    xqch = []
                    for half in range(2):
                        xq_h = xqc_p.tile([128, 3, XC], F32R, tag=f"xqc{half}")
                        dma_in(
                            xq_h[:],
                            xqt_d[
                                half * 384 : (half + 1) * 384,
                                sc * XC : (sc + 1) * XC,
                            ].rearrange("(ko p) s -> p ko s", p=128),
                        )
                        xqch.append(xq_h)
                    if sc == 0:
                        # prefetch W_k during the Q phase
                        for half in range(2):
                            dma_in(
                                wtk[:, half * 3 : (half + 1) * 3, :],
                                wkt_d[half * 384 : (half + 1) * 384, :].rearrange(
                                    "(ko p) o -> p ko o", p=128
                                ),
                            )
                    for oo in range(DK):
                        pq = ps_q.tile([128, XC], F32, tag="ps_q")
                        for ko in range(DK):
                            nc.tensor.matmul(
                                pq[:],
                                wtqh[ko // 3][:, ko % 3, oo * 128 : (oo + 1) * 128],
                                xqch[ko // 3][:, ko % 3, :],
                                start=(ko == 0),
                                stop=(ko == DK - 1),
                            )
                        nc.vector.tensor_copy(qt_sb[:, oo, sc * XC : (sc + 1) * XC], pq[:])

            # ---------------- Phase 1a: K^T / V projections ----------------
            with ExitStack() as p2:
                wvpool = p2.enter_context(tc.tile_pool(name="wvpool", bufs=1))
                xtc_p = p2.enter_context(tc.tile_pool(name="xtc", bufs=2))
                ps_p512 = p2.enter_context(
                    tc.tile_pool(name="ps_p512", bufs=3, space="PSUM")
                )
                ps_p384 = p2.enter_context(
                    tc.tile_pool(name="ps_p384", bufs=4, space="PSUM")
                )

                wtv = wvpool.tile([128, DK, D], F32R, name="wtv")

                for sc in range(SEQ // XC):
                    xtc = xtc_p.tile([128, DK, XC], F32R, tag="xtc")
                    for half in range(2):
                        dma_in(
                            xtc[:, half * 3 : (half + 1) * 3, :],
                            xt_d[
                                half * 384 : (half + 1) * 384,
                                sc * XC : (sc + 1) * XC,
                            ].rearrange("(ko p) s -> p ko s", p=128),
                        )
                    if sc == 0:
                        # W_v load queues behind xtc0 so K-chunk0 starts sooner
                        for half in range(2):
                            dma_in(
                                wtv[:, half * 3 : (half + 1) * 3, :],
                                wvt_d[half * 384 : (half + 1) * 384, :].rearrange(
                                    "(ko p) o -> p ko o", p=128
                                ),
                            )

                    # K^T chunk
                    for oo in range(DK):
                        pk = ps_p512.tile([128, XC], F32, tag="p512")
                        for ko in range(DK):
                            nc.tensor.matmul(
                                pk[:],
                                wtk[:, ko, oo * 128 : (oo + 1) * 128],
                                xtc[:, ko, :],
                                start=(ko == 0),
                                stop=(ko == DK - 1),
                            )
                        evac(kt[:, oo, sc * XC : (sc + 1) * XC], pk[:])

                    # V chunk: per 128-row seq tile, dout in two 384 halves
                    for st in range(XC // 128):
                        seq_tile = sc * (XC // 128) + st
                        for oc in range(2):
                            pv = ps_p384.tile([128, 384], F32, tag="p384")
                            for ko in range(DK):
                                nc.tensor.matmul(
                                    pv[:],
                                    xtc[:, ko, st * 128 : (st + 1) * 128],
                                    wtv[:, ko, oc * 384 : (oc + 1) * 384],
                                    start=(ko == 0),
                                    stop=(ko == DK - 1),
                                )
                            evac(vt[:, seq_tile, oc * 384 : (oc + 1) * 384], pv[:])

            # ---------------- Phase 2: attention per local q-tile ----------------
            with ExitStack() as p3:
                scd_p = p3.enter_context(tc.tile_pool(name="scd", bufs=3))
                attn_p = p3.enter_context(tc.tile_pool(name="attn", bufs=3))
                attnT_p = p3.enter_context(tc.tile_pool(name="attnT", bufs=3))
                ctx_p = p3.enter_context(tc.tile_pool(name="ctxs", bufs=3))
                small_p = p3.enter_context(tc.tile_pool(name="small", bufs=2))
                ps_s = p3.enter_context(tc.tile_pool(name="ps_s", bufs=3, space="PSUM"))
                ps_t3 = p3.enter_context(
                    tc.tile_pool(name="ps_t3", bufs=2, space="PSUM")
                )
                ps_c1 = p3.enter_context(
                    tc.tile_pool(name="ps_c1", bufs=2, space="PSUM")
                )
                ps_c2 = p3.enter_context(
                    tc.tile_pool(name="ps_c2", bufs=1, space="PSUM")
                )

                for lt in range(LT):
                    ncs = NCS[lt]
                    attn = attn_p.tile([128, SEQ], F32, tag="attn")
                    rs = small_p.tile([128, 4], F32, tag="rs")

                    for kc in range(ncs):
                        pss = ps_s.tile([128, 512], F32, tag="ps_s")
                        for ko in range(DK):
                            nc.tensor.matmul(
                                pss[:],
                                qt_sb[:, ko, lt * 128 : (lt + 1) * 128],
                                kt[:, ko, kc * 512 : (kc + 1) * 512],
                                start=(ko == 0),
                                stop=(ko == DK - 1),
                            )
                        if kc == ncs - 1:
                            # diagonal chunk: strip-add (VectorE, fused evac), then exp
                            scd = scd_p.tile([128, 512], F32, tag="scd")
                            nc.vector.tensor_add(
                                scd[:],
                                pss[:],
                                strip[:, (lt % 2) * 512 : (lt % 2) * 512 + 512],
                            )
                            nc.scalar.activation(
                                attn[:, kc * 512 : (kc + 1) * 512],
                                scd[:],
                                mybir.ActivationFunctionType.Exp,
                                scale=SCALE,
                                accum_out=rs[:, kc : kc + 1],
                            )
                        else:
                            # interior chunk: exp straight from PSUM
                            nc.scalar.activation(
                                attn[:, kc * 512 : (kc + 1) * 512],
                                pss[:],
                                mybir.ActivationFunctionType.Exp,
                                scale=SCALE,
                                accum_out=rs[:, kc : kc + 1],
                            )

                    attnT = attnT_p.tile([128, SEQ // 128, 128], F32R, tag="attnT")
                    for kc in range(ncs):
                        pst = ps_t3.tile([128, 512], F32, tag="ps_t3")
                        for t in range(4):
                            nc.tensor.matmul(
                                pst[:, t * 128 : (t + 1) * 128],
                                attn[:, (kc * 4 + t) * 128 : (kc * 4 + t + 1) * 128],
                                ident[:],
                                is_transpose=True,
                                start=(t == 0),
                                stop=(t == 3),
                            )
                        nc.vector.tensor_copy(attnT[:, kc * 4 : kc * 4 + 4, :], pst[:])

                    nkt = 4 * ncs
                    pc1 = ps_c1.tile([128, 512], F32, tag="ps_c1")
                    pc2 = ps_c2.tile([128, 256], F32, tag="ps_c2")
                    for ktile in range(nkt):
                        nc.tensor.matmul(
                            pc1[:],
                            attnT[:, ktile, :],
                            vt[:, ktile, 0:512],
                            start=(ktile == 0),
                            stop=(ktile == nkt - 1),
                        )
                    for ktile in range(nkt):
                        nc.tensor.matmul(
                            pc2[:],
                            attnT[:, ktile, :],
                            vt[:, ktile, 512:768],
                            start=(ktile == 0),
                            stop=(ktile == nkt - 1),
                        )

                    rsum = small_p.tile([128, 1], F32, tag="rsum")
                    nc.vector.reduce_sum(
                        rsum[:], rs[:, 0:ncs], axis=mybir.AxisListType.X
                    )
                    rinv = small_p.tile([128, 1], F32, tag="rinv")
                    nc.vector.reciprocal(rinv[:], rsum[:])

                    ctx_sb = ctx_p.tile([128, D], F32, tag="ctxs")
                    nc.vector.tensor_mul(
                        ctx_sb[:, 0:512], pc1[:], rinv[:].to_broadcast((128, 512))
                    )
                    nc.vector.tensor_mul(
                        ctx_sb[:, 512:768], pc2[:], rinv[:].to_broadcast((128, 256))
                    )
                    nc.sync.dma_start(out_d[lt * 128 : (lt + 1) * 128, :], ctx_sb[:])

    nc.compile()
    return nc


def _strip_variant(v):
    """Within-chunk causal mask for a diagonal chunk of residue v = g mod 4:
    allow key jj (0..511) for row i iff jj <= 128*v + i."""
    i = np.arange(128)[:, None]
    jj = np.arange(512)[None, :]
    return np.where(jj <= 128 * v + i, 0.0, NEG).astype(np.float32)


def kernel(x, Wq, Wk, Wv):
    if "nc" not in _CACHE:
        _CACHE["nc"] = _build()
    nc = _CACHE["nc"]

    x = np.ascontiguousarray(x, dtype=np.float32)
    wqt = np.ascontiguousarray(np.asarray(Wq, dtype=np.float32).T)
    wkt = np.ascontiguousarray(np.asarray(Wk, dtype=np.float32).T)
    wvt = np.ascontiguousarray(np.asarray(Wv, dtype=np.float32).T)

    in_maps = []
    for c in range(8):
        b, h = c // 2, c % 2
        xb = x[b]
        # own query rows: global q-tiles 2*lt + h
        own = np.concatenate(
            [xb[(2 * lt + h) * 128 : (2 * lt + h + 1) * 128] for lt in range(LT)],
            axis=0,
        )
        # strip windows: lt%2==0 -> variant h; lt%2==1 -> variant 2+h
        strip = np.concatenate([_strip_variant(h), _strip_variant(2 + h)], axis=1)
        in_maps.append(
            {
                "xt": np.ascontiguousarray(xb.T),
                "xqt": np.ascontiguousarray(own.T),
                "wqt": wqt,
                "wkt": wkt,
                "wvt": wvt,
                "strip": np.ascontiguousarray(strip),
            }
        )

    res = run_bass_kernel_spmd(
        nc,
        in_maps,
        list(range(8)),
        trace=bool(int(os.environ.get("KERNEL_TRACE", "0"))),
    )
    _CACHE["last_results"] = res

    out = np.empty((BATCH, SEQ, D), np.float32)
    for c in range(8):
        b, h = c // 2, c % 2
        o = res.results[c]["out"]
        for lt in range(LT):
            out[b, (2 * lt + h) * 128 : (2 * lt + h + 1) * 128] = o[
                lt * 128 : (lt + 1) * 128
            ]
    return out

